# revision 7
# baseline (speedup 1.0000x reference)
"""Baichuan paged-attention layer on 8 trn2 cores, tensor-parallel over heads.

Per core c: heads 4c..4c+3. Device computes QKV proj, RoPE, attention vs
[gathered history KV + new KV], and a partial o_proj [T, HID] against
w_o[:, 512c:512c+512]. Host gathers history KV pages, builds RoPE/mask
tables, and sums the 8 partial outputs (bf16 partials, f64 accumulate).
All matmul operands are bf16 (fp32 PSUM accumulation); softmax/RoPE
arithmetic stays fp32 on the vector/scalar engines.

Sequences are processed in two groups of two: the Q/K weight stream is
shared across the group (halves wq DMA), V-projection chunks interleave
between rt sweeps, o_proj weights are prefetched at kernel start, and each
sequence's o_proj partial is emitted right after its attention so the PE
stays dense across phase boundaries.
"""
import sys

sys.path.insert(0, "/opt/trn_rl_repo")
import numpy as np

H = 32; D = 128; HID = 4096; BS = 64; NBLOCKS = 128
B = 4; QLEN = 512; MAXBLK = 24; ROPE_BASE = 10000.0
T = B * QLEN; NCORES = 8; HC = H // NCORES; W = HC * D  # 4 heads, 512 wide
NEG = -1.0e30
SCALE = 1.0 / float(np.sqrt(D))

_cache = {}
last_results = None  # BassKernelResults of the most recent run (for test.py)

# pool sizing knobs
BUFS = dict(cs=2, hid=8, wq=4, wv=3, qkr=16, qs=2, ropet=2, vsb=8,
            kh=2, vh=2, exp=3, smol=1, stg=4, attn=16, wo=8)


def _round128(x):
    return (x + 127) // 128 * 128


def _np_bf16():
    from concourse import mybir
    return mybir.dt.np(mybir.dt.bfloat16)


def _build(hist):
    import concourse.bass as bass
    import concourse.tile as tile
    from concourse import bacc, mybir

    F32 = mybir.dt.float32
    BF16 = mybir.dt.bfloat16
    np_bf16 = _np_bf16()

    hv = [_round128(h) for h in hist]
    SH = [x // 128 for x in hv]

    nc = bacc.Bacc("TRN2", target_bir_lowering=False, debug=False,
                   num_devices=NCORES)
    hiddenT_d = nc.dram_tensor("hiddenT", [HID, T], BF16, kind="ExternalInput")
    # wql: [p, (rt*4+kc)*1024 + s*128 + c] = wqk[rt*128+c, kc*1024+s*128+p]
    wql_d = nc.dram_tensor("wql", [128, 8 * 4 * 1024], BF16,
                           kind="ExternalInput")
    wvT_d = nc.dram_tensor("wvT", [HID, W], BF16, kind="ExternalInput")
    woT_d = nc.dram_tensor("woT", [W, HID], BF16, kind="ExternalInput")
    kh_d = [nc.dram_tensor(f"khT{b}", [W, hv[b]], BF16, kind="ExternalInput")
            if hv[b] else None for b in range(B)]
    vh_d = [nc.dram_tensor(f"vh{b}", [hv[b], W], BF16, kind="ExternalInput")
            if hv[b] else None for b in range(B)]
    out_d = nc.dram_tensor("out", [T, HID], BF16, kind="ExternalOutput")

    # host-built tables baked into the NEFF
    inv = 1.0 / (ROPE_BASE ** (np.arange(0, D, 2) / D))
    pos = np.concatenate([h + np.arange(QLEN) for h in hist]).astype(np.float64)
    ang = np.concatenate([inv, inv])[:, None] * pos[None, :]
    cos_d = nc.inline_tensor(np.cos(ang).astype(np.float32), name="cosT")
    sin_d = nc.inline_tensor(np.sin(ang).astype(np.float32), name="sinT")

    mask_np = np.where(
        np.arange(128)[:, None] <= np.arange(896)[None, :] - 384,
        0.0, NEG).astype(np.float32)
    mask_d = nc.inline_tensor(mask_np, name="maskS")

    pad_np = np.zeros((128, B), np.float32)
    for b in range(B):
        if hv[b]:
            pad_np[:, b] = np.where(hv[b] - 128 + np.arange(128) >= hist[b],
                                    NEG, 0.0)
    pad_d = nc.inline_tensor(pad_np, name="padc")

    Pm = np.zeros((128, 128), np.float32)
    for d in range(64):
        Pm[d, d + 64] = -1.0
        Pm[d + 64, d] = 1.0
    pt_d = nc.inline_tensor(np.ascontiguousarray(Pm.T).astype(np_bf16),
                            name="permT")
    ones_d = nc.inline_tensor(np.ones((128, 1), np_bf16), name="ones")

    from contextlib import ExitStack

    with tile.TileContext(nc) as tc:
        with ExitStack() as ctx:
            cpool = ctx.enter_context(tc.tile_pool(name="const", bufs=1))
            apool = ctx.enter_context(
                tc.tile_pool(name="attn", bufs=BUFS["attn"]))
            wopool = ctx.enter_context(
                tc.tile_pool(name="wop", bufs=BUFS["wo"]))
            pspool = ctx.enter_context(
                tc.tile_pool(name="psum", bufs=8, space="PSUM"))
            mask_t = cpool.tile([128, 896], F32, tag="mask")
            nc.sync.dma_start(mask_t[:], mask_d[:])
            pad_t = cpool.tile([128, B], F32, tag="pad")
            nc.sync.dma_start(pad_t[:], pad_d[:])
            pt_t = cpool.tile([128, 128], BF16, tag="pt")
            nc.sync.dma_start(pt_t[:], pt_d[:])
            ones_t = cpool.tile([128, 1], BF16, tag="ones")
            nc.sync.dma_start(ones_t[:], ones_d[:])

            # prefetch all o_proj weights up front
            wots = []
            for ic in range(8):
                isl = slice(ic * 512, (ic + 1) * 512)
                wot = wopool.tile([128, 4, 512], BF16, tag="wo",
                                  name=f"wot{ic}")
                nc.sync.dma_start(
                    wot[:],
                    woT_d[:, isl].rearrange("(s p) c -> p s c", p=128))
                wots.append(wot)

            cspool = ctx.enter_context(tc.tile_pool(name="cs", bufs=BUFS["cs"]))
            hidpool = ctx.enter_context(
                tc.tile_pool(name="hid", bufs=BUFS["hid"]))
            wqpool = ctx.enter_context(
                tc.tile_pool(name="wst", bufs=BUFS["wq"]))
            wvpool = ctx.enter_context(
                tc.tile_pool(name="wvst", bufs=BUFS["wv"]))
            qkrpool = ctx.enter_context(
                tc.tile_pool(name="qkr", bufs=BUFS["qkr"]))
            rppool = ctx.enter_context(
                tc.tile_pool(name="rope", bufs=BUFS["qs"]))
            vpool = ctx.enter_context(
                tc.tile_pool(name="vsb", bufs=BUFS["vsb"]))
            khpool = ctx.enter_context(
                tc.tile_pool(name="khp", bufs=BUFS["kh"]))
            vhpool = ctx.enter_context(
                tc.tile_pool(name="vhp", bufs=BUFS["vh"]))
            epool = ctx.enter_context(
                tc.tile_pool(name="expp", bufs=BUFS["exp"]))
            smpool = ctx.enter_context(
                tc.tile_pool(name="smol", bufs=BUFS["smol"]))
            stpool = ctx.enter_context(
                tc.tile_pool(name="stg", bufs=BUFS["stg"]))
            if True:
                for g in range(2):
                    bs = (2 * g, 2 * g + 1)
                    cos_t, sin_t, hid_c = {}, {}, {}
                    for b in bs:
                        tsl = slice(b * QLEN, (b + 1) * QLEN)
                        cos_t[b] = cspool.tile([128, QLEN], F32, tag="cos",
                                               name=f"cos{b}")
                        nc.sync.dma_start(cos_t[b][:], cos_d[:, tsl])
                        sin_t[b] = cspool.tile([128, QLEN], F32, tag="sin",
                                               name=f"sin{b}")
                        nc.sync.dma_start(sin_t[b][:], sin_d[:, tsl])
                        hid_c[b] = []
                        for kc in range(4):
                            ht = hidpool.tile([128, 8, QLEN], BF16, tag="hid",
                                              name=f"hid{b}_{kc}")
                            nc.sync.dma_start(
                                ht[:],
                                hiddenT_d[kc * 1024:(kc + 1) * 1024, tsl]
                                .rearrange("(s p) t -> p s t", p=128))
                            hid_c[b].append(ht)

                    # history V prefetch for both sequences of the group
                    vht = {}
                    for b in bs:
                        if SH[b]:
                            vht[b] = vhpool.tile([128, 8, W], BF16, tag="vh",
                                                 name=f"vh_t{b}")
                            nc.sync.dma_start(
                                vht[b][:, :SH[b], :],
                                vh_d[b][:].rearrange("(s p) c -> p s c",
                                                     p=128))

                    # ---- V projection first (per seq; wv streamed twice)
                    v_sb = {}
                    for b in bs:
                        v_sb[b] = [vpool.tile([128, W], BF16, tag="vsb",
                                              name=f"vsb{b}_{i}")
                                   for i in range(4)]
                        v_ps = [pspool.tile([128, W], F32, tag="ps",
                                            name=f"vps{b}_{i}")
                                for i in range(4)]
                        for kc2 in range(16):
                            wvt = wvpool.tile([128, 2, W], BF16, tag="wv")
                            nc.sync.dma_start(
                                wvt[:],
                                wvT_d[kc2 * 256:(kc2 + 1) * 256, :]
                                .rearrange("(s p) c -> p s c", p=128))
                            for s2 in range(2):
                                k = kc2 * 2 + s2
                                for tt in range(4):
                                    nc.tensor.matmul(
                                        v_ps[tt][:],
                                        hid_c[b][k // 8][:, k % 8,
                                                         tt * 128:(tt + 1) * 128],
                                        wvt[:, s2, :],
                                        start=(k == 0), stop=(k == 31))
                        for tt in range(4):
                            nc.vector.tensor_copy(v_sb[b][tt][:], v_ps[tt][:])

                    # ---- QK proj (wq shared across the group) + RoPE, one
                    # head pair (Q_h, K_h) at a time; each head's attention
                    # runs as soon as its pair lands.
                    qk_rot = {b: {} for b in bs}
                    attn_b = {b: [] for b in bs}
                    for hp in range(4):
                        for rt in (hp, 4 + hp):
                            pq = {}
                            for b in bs:
                                pq[b] = pspool.tile([128, QLEN], F32,
                                                    tag="ps", name=f"pq{b}")
                            for kc in range(4):
                                wqt = wqpool.tile([128, 1024], BF16, tag="wq")
                                nc.sync.dma_start(
                                    wqt[:],
                                    wql_d[:, (rt * 4 + kc) * 1024:
                                          (rt * 4 + kc + 1) * 1024])
                                for s in range(8):
                                    for b in bs:
                                        nc.tensor.matmul(
                                            pq[b][:],
                                            wqt[:, s * 128:(s + 1) * 128],
                                            hid_c[b][kc][:, s, :],
                                            start=(kc == 0 and s == 0),
                                            stop=(kc == 3 and s == 7))
                            for b in bs:
                                qs = rppool.tile([128, QLEN], BF16, tag="qs")
                                nc.scalar.copy(qs[:], pq[b][:])
                                rot = pspool.tile([128, QLEN], F32, tag="ps")
                                nc.tensor.matmul(rot[:], pt_t[:], qs[:],
                                                 start=True, stop=True)
                                t1 = rppool.tile([128, QLEN], F32, tag="t1",
                                                 bufs=BUFS["ropet"])
                                nc.vector.tensor_mul(t1[:], rot[:],
                                                     sin_t[b][:])
                                t2 = rppool.tile([128, QLEN], F32, tag="t2",
                                                 bufs=BUFS["ropet"])
                                nc.vector.tensor_mul(t2[:], pq[b][:],
                                                     cos_t[b][:])
                                qr = qkrpool.tile([128, QLEN], BF16,
                                                  tag="qkr")
                                nc.vector.tensor_add(qr[:], t1[:], t2[:])
                                qk_rot[b][rt] = qr

                        # attention for head hp, both sequences
                        h = hp
                        for b in bs:
                            S = SH[b] + 4
                            kh_t = None
                            if SH[b]:
                                kh_t = khpool.tile([128, hv[b]], BF16,
                                                   tag="kh")
                                nc.sync.dma_start(
                                    kh_t[:], kh_d[b][h * 128:(h + 1) * 128, :])
                            dn = pspool.tile([1, QLEN], F32, tag="ps")
                            pv = pspool.tile([128, QLEN], F32, tag="ps")
                            for st in range(S):
                                sc = pspool.tile([128, QLEN], F32, tag="ps")
                                if st < SH[b]:
                                    lhsT = kh_t[:, st * 128:(st + 1) * 128]
                                else:
                                    j = st - SH[b]
                                    lhsT = qk_rot[b][4 + h][:, j * 128:
                                                            (j + 1) * 128]
                                nc.tensor.matmul(sc[:], lhsT, qk_rot[b][h][:],
                                                 start=True, stop=True)
                                if st == SH[b] - 1 and hist[b] != hv[b]:
                                    nc.vector.tensor_scalar_add(
                                        sc[:], sc[:], pad_t[:, b:b + 1])
                                if st >= SH[b]:
                                    j = st - SH[b]
                                    nc.vector.tensor_add(
                                        sc[:], sc[:],
                                        mask_t[:, 384 - 128 * j:896 - 128 * j])
                                ex = epool.tile([128, QLEN], BF16, tag="exp")
                                nc.scalar.activation(
                                    ex[:], sc[:],
                                    mybir.ActivationFunctionType.Exp,
                                    scale=SCALE)
                                nc.tensor.matmul(dn[:], ones_t[:], ex[:],
                                                 start=(st == 0),
                                                 stop=(st == S - 1))
                                if st < SH[b]:
                                    vt = vht[b][:, st, h * 128:(h + 1) * 128]
                                else:
                                    vt = v_sb[b][st - SH[b]][:, h * 128:
                                                             (h + 1) * 128]
                                nc.tensor.matmul(pv[:], vt, ex[:],
                                                 start=(st == 0),
                                                 stop=(st == S - 1))
                            rc = smpool.tile([1, QLEN], F32, tag="rc")
                            nc.vector.reciprocal(rc[:], dn[:])
                            bcs = smpool.tile([128, QLEN], F32, tag="bcs")
                            nc.gpsimd.partition_broadcast(bcs[:], rc[:])
                            at = apool.tile([128, QLEN], BF16, tag="attn")
                            nc.vector.tensor_mul(at[:], pv[:], bcs[:])
                            attn_b[b].append(at)

                    # ---- o_proj partials for the group
                    for b in bs:
                        for ic in range(8):
                            isl = slice(ic * 512, (ic + 1) * 512)
                            for q in range(4):
                                tt = b * 4 + q
                                po = pspool.tile([128, 512], F32, tag="ps")
                                for jt in range(4):
                                    nc.tensor.matmul(
                                        po[:],
                                        attn_b[b][jt][:, q * 128:(q + 1) * 128],
                                        wots[ic][:, jt, :],
                                        start=(jt == 0), stop=(jt == 3))
                                st_ = stpool.tile([128, 512], BF16, tag="stg")
                                if (ic + q) % 2 == 0:
                                    nc.vector.tensor_copy(st_[:], po[:])
                                else:
                                    nc.scalar.copy(st_[:], po[:])
                                nc.sync.dma_start(
                                    out_d[tt * 128:(tt + 1) * 128, isl],
                                    st_[:])
    nc.compile()
    return {"nc": nc}


def _get(hist):
    if hist not in _cache:
        _cache[hist] = _build(hist)
    return _cache[hist]


def prepare_in_maps(inputs):
    np_bf16 = _np_bf16()
    hidden = np.asarray(inputs["hidden_states"], np.float32)
    w_pack = np.asarray(inputs["w_pack"], np.float32)
    w_o = np.asarray(inputs["w_o"], np.float32)
    kc = np.asarray(inputs["key_cache"], np.float32).reshape(NBLOCKS * BS, H, D)
    vc = np.asarray(inputs["value_cache"], np.float32).reshape(NBLOCKS * BS, H, D)
    bo = np.asarray(inputs["block_offsets"], np.int32)
    hist = tuple(int(x) for x in np.asarray(inputs["history_lengths"]))
    assert all(0 <= h and h + QLEN <= MAXBLK * BS for h in hist)
    hv = [_round128(h) for h in hist]

    built = _get(hist)
    hiddenT = np.ascontiguousarray(hidden.T).astype(np_bf16)

    in_maps = []
    for c in range(NCORES):
        rs = slice(c * W, (c + 1) * W)
        wqk = np.concatenate(
            [w_pack[rs], w_pack[HID + c * W:HID + (c + 1) * W]], axis=0)
        # wql[p, rt, kc, s, c] = wqk[rt*128+c, kc*1024+s*128+p]
        wql = np.ascontiguousarray(
            wqk.reshape(8, 128, 4, 8, 128).transpose(4, 0, 2, 3, 1)
            .reshape(128, 8 * 4 * 1024)).astype(np_bf16)
        wv = w_pack[2 * HID + c * W:2 * HID + (c + 1) * W]
        im = {
            "hiddenT": hiddenT,
            "wql": wql,
            "wvT": np.ascontiguousarray(wv.T).astype(np_bf16),
            "woT": np.ascontiguousarray(w_o[:, rs].T).astype(np_bf16),
        }
        for b in range(B):
            if not hv[b]:
                continue
            nblk = (hist[b] + BS - 1) // BS
            rows = (bo[b, :nblk, None] * BS +
                    np.arange(BS)[None, :]).reshape(-1)[:hist[b]]
            khp = np.zeros((hv[b], HC, D), np.float32)
            khp[:hist[b]] = kc[rows][:, c * HC:(c + 1) * HC, :]
            vhp = np.zeros((hv[b], HC, D), np.float32)
            vhp[:hist[b]] = vc[rows][:, c * HC:(c + 1) * HC, :]
            im[f"khT{b}"] = np.ascontiguousarray(
                khp.transpose(1, 2, 0).reshape(W, hv[b])).astype(np_bf16)
            im[f"vh{b}"] = np.ascontiguousarray(
                vhp.reshape(hv[b], W)).astype(np_bf16)
        in_maps.append(im)
    return built["nc"], in_maps


def kernel(**inputs):
    global last_results
    from concourse.bass_utils import run_bass_kernel_spmd

    nc, in_maps = prepare_in_maps(inputs)
    last_results = run_bass_kernel_spmd(nc, in_maps,
                                        core_ids=list(range(NCORES)))
    acc = np.zeros((T, HID), np.float64)
    for c in range(NCORES):
        acc += last_results.results[c]["out"].astype(np.float64)
    return acc.astype(np.float32)
